# revision 34
# baseline (speedup 1.0000x reference)
"""Trainium2 Bass kernel for nn_Attention_75299366633572 (v3).

Math (reference):
    scale[s] = temporal-PE flattened, s in [0, 1024)
    xs[n,s,:] = x[n,s,:] * scale[s]
    h = xs @ W.T + b                       # [N, S, 384]
    q,k,v = interleaved split of h         # each [N, S*128] via h[...,0::3] etc.
    scores = q @ k.T / sqrt(128)           # [128, 128]  (attention over batch!)
    out = softmax(scores) @ v              # [128, 131072]

Algebraic restructure (per position s, with Wq' = Wq/sqrt(128)):
    scores[n,m] = sum_s xs_s[n,:] A xs_s[m,:].T + (w . xs_s[m,:]) + rowconst
        A = Wq'.T @ Wk   [128,128],   w = Wk.T @ bq'
    row-constant terms are softmax-invariant -> dropped.
    v bias: softmax rows sum to 1 -> bv added on host at the end.

v4 design (vs the 144us v2 baseline; measures ~95-120us, the spread is
run-to-run launch/collective jitter):
  * fp16 datapath (validated on host: rel err 5.8e-3 vs the 2e-2 budget).
    Halves the XT DMA (4 MiB/core) and makes every matmul single-pass.
  * scores accumulated TRANSPOSED (scT[m,n] += XT_s-stationary @ yt_s):
    the V matmul shares the same stationary XT_s, so one LDWEIGHTS feeds
    both the score and the V matmul (LDWEIGHTS serializes with matmul on
    TRN2 -- it was ~30% of the baseline's phase-1 time).  The whole
    Y/scores/V computation is ONE fused streaming sweep over XT, done by
    ~45us.
  * software-pipelined sweep: Y(c+1) is emitted before the score/V loop
    of chunk c so the PE never waits on the DVE/ACT yt drain.
  * AllReduce(add) of the [128,128] partial scores instead of
    AllGather + 3 gpsimd tree adds + 512 KiB strided readback.
    NOTE the hard floor: NRT's collectives-init barrier occupies the CC
    cores until ~55-60us into the NEFF regardless of kernel content, and
    any collective pays ~11us of mesh handshake + ~14us transfer after
    it.  The sweep runs entirely in that shadow; nothing in the kernel
    waits on the collective until it truly must.  (A CC-free exchange via
    SWDGE remote_dma_broadcast was prototyped to dodge the barrier, but
    it has no Tile/scheduler support and hangs on this runtime.)
  * softmax with NO transposes and NO row-max: exp is applied in the scT
    [m,n] layout with a softmax-invariant constant -40 bias (logits here
    are < ~75, so f32/bf16 exp cannot overflow), producing the
    unnormalized exp^T in bf16 which IS the out-matmul stationary (bf16
    keeps the full f32 exponent range, unlike fp16).  Row sums Z[n] come
    from one ones-vector matmul (same stationary, contraction over m) and
    1/Z folds into the per-partition scale of the output drains.  The
    post-collective critical path is readback -> exp -> matmul, ~3us.
  * output DMAs ride the sync + gpsimd queues only: a dma_start costs
    ~700ns of issue time on its engine's stream, which starves the ACT
    drain stream if placed there.

Sharding: S (sequence) split across 8 cores (128 positions each); each
core emits output columns for its own S-shard.
"""

import math

import numpy as np

import concourse.bass as bass
import concourse.mybir as mybir
import concourse.tile as tile
from concourse import bacc
from concourse.bass_utils import run_bass_kernel_spmd
from concourse.masks import make_identity

NCORES = 8
N = 128            # batch rows (attention is over this axis)
S = 1024           # sequence positions
D = 128            # feature dim
S_LOC = S // NCORES       # 128 positions per core
COLS = S_LOC * D          # 16384 free columns per core
NCH = S_LOC // 4          # 32 sweep chunks of 512 cols (4 positions)
VTAIL = 0                 # V fully fused into the sweep (the collective's
                          # start is gated by the NRT init barrier ~60us in,
                          # far after the sweep ends -- nothing to hide)
F32 = mybir.dt.float32
F32R = mybir.dt.float32r
F16 = mybir.dt.float16
BF16 = mybir.dt.bfloat16

_CACHE = {}


def _temporal_scale():
    """pe.flatten() from the reference's _temporal_pe, float32."""
    i = np.arange(32, dtype=np.float32)[:, None]
    j = np.arange(16, dtype=np.float32)[None, :]
    arg = (np.float32(1.0) * np.float32(np.pi) * i
           / np.power(np.float32(1000.0), (np.float32(2.0) * j / np.float32(128.0))))
    pe = np.stack([np.sin(arg), np.cos(arg)], axis=-1).reshape(32, 32)
    return pe.reshape(-1).astype(np.float32)   # [1024]


def _emit(nc, tc, xt_d, A_d, w_d, WvT_d, out_d):
    AX = mybir.AxisListType
    AF = mybir.ActivationFunctionType
    NFUSED = S_LOC - VTAIL          # positions with V fused into the sweep

    with (
        tc.tile_pool(name="consts", bufs=1) as consts,
        tc.tile_pool(name="xt", bufs=1) as xtp,
        tc.tile_pool(name="vbuf", bufs=1) as vp,
        tc.tile_pool(name="small", bufs=1) as small,
        tc.tile_pool(name="dram", bufs=1, space="DRAM") as dram,
    ):
        ident = consts.tile([128, 128], F32)
        make_identity(nc, ident[:])
        A_sb = consts.tile([D, D], F16)
        nc.sync.dma_start(A_sb[:], A_d[:])
        w_sb = consts.tile([D, 1], F32)
        nc.sync.dma_start(w_sb[:], w_d[:])
        WvT_sb = consts.tile([D, D], F16)
        nc.sync.dma_start(WvT_sb[:], WvT_d[:])

        XT = xtp.tile([128, COLS], F16)      # xs^T, [d, (s,n)]
        V = vp.tile([128, COLS], BF16)       # v rows, [m, (s,g)]

        scT_sb = small.tile([128, 128], F32, tag="scT")
        ar_sb = small.tile([128, 128], F32, tag="ar")
        exT = small.tile([128, 128], BF16, tag="exT")
        ones = small.tile([128, 2], BF16, tag="ones")
        rinv = small.tile([128, 1], F32, tag="rinv")
        nbias = small.tile([128, 1], F32, tag="nbias")
        nc.gpsimd.memset(ones[:], 1.0)
        nc.gpsimd.memset(nbias[:], -40.0)

        in_b = dram.tile([128, 128], F32)
        out_b = dram.tile([128, 128], F32)

        # XT input on one HWDGE queue, ascending so Y(0) starts early.
        bounds = [0, 128, 256, 512, 1024] + [1024 * i for i in range(2, 17)]
        for lo, hi in zip(bounds[:-1], bounds[1:]):
            nc.sync.dma_start(XT[:, lo:hi], xt_d[:, lo:hi])

        # Warm-up: PE clock gate starts at 1.2 GHz; burn transposes inside
        # the first-chunk DMA wait so the sweep starts warm.
        with tc.tile_pool(name="ps_wu", bufs=1, space="PSUM") as ps_wu:
            wps = ps_wu.tile([128, 128], F32)
            for _ in range(16):
                nc.tensor.transpose(wps[:], ident[:], ident[:])

        # ---- Sweep: Y = A^T@XT (+w), scT += XT_s^T@yt_s, V_s = XT_s^T@WvT
        with (
            tc.tile_pool(name="yt", bufs=3) as ytp,
            tc.tile_pool(name="ps_y", bufs=3, space="PSUM") as ps_y,
            tc.tile_pool(name="ps_v", bufs=2, space="PSUM") as ps_v,
            tc.tile_pool(name="ps_sc", bufs=1, space="PSUM") as ps_sc,
        ):
            sc_ps = ps_sc.tile([128, 128], F32)

            def emit_y(c):
                yps = ps_y.tile([128, 512], F32, tag="y")
                nc.tensor.matmul(yps[:], A_sb[:], XT[:, c * 512:(c + 1) * 512],
                                 start=True, stop=True)
                yt = ytp.tile([128, 512], F16, tag="yt")
                if c % 2 == 0:
                    nc.vector.tensor_scalar_add(yt[:], yps[:], w_sb[:, 0:1])
                else:
                    nc.scalar.add(yt[:], yps[:], w_sb[:, 0:1])
                return yt

            pending = emit_y(0)
            for c in range(NCH):
                yt = pending
                if c + 1 < NCH:
                    pending = emit_y(c + 1)
                vps = (ps_v.tile([128, 512], F32, tag="v", name="vps")
                       if 4 * c < NFUSED else None)
                for k in range(4):
                    s = 4 * c + k
                    xs_s = XT[:, s * 128:(s + 1) * 128]
                    nc.tensor.matmul(sc_ps[:], xs_s, yt[:, k * 128:(k + 1) * 128],
                                     start=(s == 0), stop=(s == S_LOC - 1))
                    if vps is not None:
                        nc.tensor.matmul(vps[:, k * 128:(k + 1) * 128], xs_s,
                                         WvT_sb[:], start=True, stop=True)
                if vps is not None:
                    dst = V[:, c * 512:(c + 1) * 512]
                    if c % 2 == 0:
                        nc.scalar.copy(dst, vps[:])
                    else:
                        nc.vector.tensor_copy(dst, vps[:])
            sc_done = nc.vector.tensor_copy(scT_sb[:], sc_ps[:])

        # ---- AllReduce the partial transposed scores (64 KiB) ----
        nc.sync.dma_start(in_b[:], scT_sb[:])
        nc.gpsimd.collective_compute(
            "AllReduce", mybir.AluOpType.add,
            replica_groups=[list(range(NCORES))],
            ins=[in_b[:].opt()], outs=[out_b[:].opt()],
        )
        nc.sync.dma_start(ar_sb[:, 0:64], out_b[:, 0:64])
        nc.scalar.dma_start(ar_sb[:, 64:128], out_b[:, 64:128])

        # ---- HAM-governor coaxing during the collective dead zone.  The
        # sweep's sustained single-pass matmuls clamp the PE to k=4/8 duty
        # and the clamp persists through idle, so the out phase would start
        # at half clock.  Sustained 4-pass fp32 matmul provably (baseline
        # phase 2) moves the governor back to k=8/8: burn ~26us of dummy
        # fp32 matmuls on the otherwise-idle PE so the out phase begins at
        # full clock.  Sized to end (~70us) before the collective typically
        # completes (~80-95us).
        with (
            tc.tile_pool(name="dummy", bufs=1) as dmp,
            tc.tile_pool(name="ps_dm", bufs=1, space="PSUM") as ps_dm,
        ):
            dmv = dmp.tile([128, 512], F32, tag="dmv")
            nc.gpsimd.memset(dmv[:], 0.0)
            dps = ps_dm.tile([128, 512], F32, tag="dps")
            for i in range(30):
                dm = nc.tensor.matmul(dps[:], ident[:], dmv[:],
                                      start=True, stop=True)
                if i == 0:
                    tile.add_dep_helper(dm.ins, sc_done.ins, sync=True,
                                        reason="dummies after score drain")

        # ---- V tail: hides the collective. Pinned after the score drain so
        # the scheduler cannot front-run it into the sweep.
        with tc.tile_pool(name="ps_v2", bufs=2, space="PSUM") as ps_v2:
            for c in range(NFUSED // 4, NCH):
                vps = ps_v2.tile([128, 512], F32, tag="v2")
                for k in range(4):
                    s = 4 * c + k
                    vm = nc.tensor.matmul(vps[:, k * 128:(k + 1) * 128],
                                          XT[:, s * 128:(s + 1) * 128],
                                          WvT_sb[:], start=True, stop=True)
                    if s == NFUSED:
                        tile.add_dep_helper(vm.ins, sc_done.ins, sync=True,
                                            reason="V tail after score drain")
                dst = V[:, c * 512:(c + 1) * 512]
                if c % 2 == 0:
                    nc.vector.tensor_copy(dst, vps[:])
                else:
                    nc.scalar.copy(dst, vps[:])

        # ---- softmax without transposes: exp stays in the scT [m, n]
        # layout (softmax-invariant constant -40 bias instead of a row-max;
        # logits for this problem are < ~75 so f32 exp cannot overflow).
        # The unnormalized exp^T IS the out-matmul stationary; the row sums
        # Z[n] come from a ones-matmul + 1-col transpose-matmul side chain
        # that overlaps the first out chunks, and 1/Z folds into the output
        # drain scaling.
        nc.scalar.activation(exT[:], ar_sb[:], AF.Exp, bias=nbias[:, 0:1],
                             scale=1.0)

        # ---- out = exT^T @ V * (1/Z), streamed to DRAM ----
        with (
            tc.tile_pool(name="osb", bufs=8) as osbp,
            tc.tile_pool(name="ps_z", bufs=1, space="PSUM") as ps_z,
            tc.tile_pool(name="ps_o", bufs=6, space="PSUM") as ps_o,
        ):
            ztp = ps_z.tile([128, 2], F32, tag="ztp")
            nc.tensor.matmul(ztp[:], exT[:], ones[:],
                             start=True, stop=True)
            nc.vector.reciprocal(rinv[:], ztp[:, 0:1])
            for c in range(NCH):
                ops = ps_o.tile([128, 512], F32, tag="o")
                nc.tensor.matmul(ops[:], exT[:], V[:, c * 512:(c + 1) * 512],
                                 start=True, stop=True)
                osb = osbp.tile([128, 512], F16, tag="osb")
                # whole-chunk drains, engines alternating by chunk: the
                # ~260ns fixed cost per drain instruction made 32 half-chunk
                # drains per engine (21.6us on ACT) the out-phase critical
                # path; 16 full-chunk drains per engine is 14.4us, back
                # under the PE.
                if c % 2 == 0:
                    nc.vector.tensor_scalar_mul(osb[:], ops[:], rinv[:, 0:1])
                else:
                    nc.scalar.mul(osb[:], ops[:], rinv[:, 0:1])
                # dma_start costs ~700ns of issue time on its engine's
                # stream, so keep output DMAs off ACT (busy with drains);
                # sync and gpsimd are otherwise idle here.
                eng = [nc.sync, nc.gpsimd][c % 2]
                eng.dma_start(out_d[:, c * 512:(c + 1) * 512], osb[:])


def _build():
    key = "v3"
    if key in _CACHE:
        return _CACHE[key]
    nc = bacc.Bacc("TRN2", target_bir_lowering=False, debug=False,
                   num_devices=NCORES)
    xt_d = nc.dram_tensor("xt", [128, COLS], F16, kind="ExternalInput")
    A_d = nc.dram_tensor("A", [D, D], F16, kind="ExternalInput")
    w_d = nc.dram_tensor("w", [D, 1], F32, kind="ExternalInput")
    WvT_d = nc.dram_tensor("WvT", [D, D], F16, kind="ExternalInput")
    out_d = nc.dram_tensor("out", [N, COLS], F16, kind="ExternalOutput")
    with tile.TileContext(nc) as tc:
        _emit(nc, tc, xt_d, A_d, w_d, WvT_d, out_d)
    nc.compile()
    _CACHE[key] = nc
    return nc


def prepare_inputs(x, W, b):
    """Host-side prep: shard + transpose x over S, build derived matrices."""
    x = np.asarray(x, dtype=np.float32)
    W = np.asarray(W, dtype=np.float32)
    b = np.asarray(b, dtype=np.float32)

    rs = math.sqrt(float(D))
    Wq = W[0::3, :].astype(np.float64) / rs
    Wk = W[1::3, :].astype(np.float64)
    Wv = W[2::3, :]
    bq = b[0::3].astype(np.float64) / rs
    bv = b[2::3]

    A = (Wq.T @ Wk).astype(np.float16)                       # [128, 128]
    w = (Wk.T @ bq).astype(np.float32)[:, None]              # [128, 1]
    WvT = np.ascontiguousarray(Wv.T).astype(np.float16)      # [128, 128]

    scale = _temporal_scale()                                # [1024]
    in_maps = []
    for c in range(NCORES):
        sl = slice(c * S_LOC, (c + 1) * S_LOC)
        xs_c = x[:, sl, :] * scale[sl][None, :, None]        # [n, s, d] f32
        xt_c = np.ascontiguousarray(
            xs_c.transpose(2, 1, 0)).reshape(D, COLS).astype(np.float16)
        in_maps.append({
            "xt": xt_c, "A": A, "w": w, "WvT": WvT,
        })
    return in_maps, bv


def run(inputs, trace=False, **kw):
    nc = _build()
    in_maps, bv = prepare_inputs(inputs["x"], inputs["W"], inputs["b"])
    res = run_bass_kernel_spmd(nc, in_maps, core_ids=list(range(NCORES)),
                               trace=trace, **kw)
    out = np.concatenate(
        [res.results[c]["out"].astype(np.float32) for c in range(NCORES)], axis=1)
    out += np.tile(bv, S)[None, :]     # v-bias: attn rows sum to 1
    return out, res


def kernel(x, W, b):
    out, _ = run({"x": x, "W": W, "b": b})
    return out


# revision 35
# speedup vs baseline: 1.0028x; 1.0028x over previous
"""Trainium2 Bass kernel for nn_Attention_75299366633572 (v3).

Math (reference):
    scale[s] = temporal-PE flattened, s in [0, 1024)
    xs[n,s,:] = x[n,s,:] * scale[s]
    h = xs @ W.T + b                       # [N, S, 384]
    q,k,v = interleaved split of h         # each [N, S*128] via h[...,0::3] etc.
    scores = q @ k.T / sqrt(128)           # [128, 128]  (attention over batch!)
    out = softmax(scores) @ v              # [128, 131072]

Algebraic restructure (per position s, with Wq' = Wq/sqrt(128)):
    scores[n,m] = sum_s xs_s[n,:] A xs_s[m,:].T + (w . xs_s[m,:]) + rowconst
        A = Wq'.T @ Wk   [128,128],   w = Wk.T @ bq'
    row-constant terms are softmax-invariant -> dropped.
    v bias: softmax rows sum to 1 -> bv added on host at the end.

v4 design (vs the 144us v2 baseline; measures ~95-110us, the spread is
run-to-run launch/collective jitter):
  * fp16 datapath (validated on host: rel err 5.8e-3 vs the 2e-2 budget).
    Halves the XT DMA (4 MiB/core) and makes every matmul single-pass.
  * scores accumulated TRANSPOSED (scT[m,n] += XT_s-stationary @ yt_s):
    the V matmul shares the same stationary XT_s, so one LDWEIGHTS feeds
    both the score and the V matmul (LDWEIGHTS serializes with matmul on
    TRN2 -- it was ~30% of the baseline's phase-1 time).  The whole
    Y/scores/V computation is ONE fused streaming sweep over XT, done by
    ~45us.
  * software-pipelined sweep: Y(c+1) is emitted before the score/V loop
    of chunk c so the PE never waits on the DVE/ACT yt drain.
  * AllReduce(add) of the [128,128] partial scores instead of
    AllGather + 3 gpsimd tree adds + 512 KiB strided readback.
    NOTE the hard floor: NRT's collectives-init barrier occupies the CC
    cores until ~55-60us into the NEFF regardless of kernel content, and
    any collective pays ~11us of mesh handshake + ~14us transfer after
    it.  The sweep runs entirely in that shadow; nothing in the kernel
    waits on the collective until it truly must.  (A CC-free exchange via
    SWDGE remote_dma_broadcast was prototyped to dodge the barrier, but
    it has no Tile/scheduler support and hangs on this runtime.)
  * softmax with NO transposes and NO row-max: exp is applied in the scT
    [m,n] layout with a softmax-invariant constant -40 bias (logits here
    are < ~75, so f32/bf16 exp cannot overflow), producing the
    unnormalized exp^T in bf16 which IS the out-matmul stationary (bf16
    keeps the full f32 exponent range, unlike fp16).  Row sums Z[n] come
    from one ones-vector matmul (same stationary, contraction over m) and
    1/Z folds into the per-partition scale of the output drains.  The
    post-collective critical path is readback -> exp -> matmul, ~3us.
  * output DMAs ride the sync + gpsimd queues only: a dma_start costs
    ~700ns of issue time on its engine's stream, which starves the ACT
    drain stream if placed there.
  * out-phase PSUM drains are whole-chunk [128,512], engines alternating
    by chunk: the ~260ns fixed cost per drain instruction made 32
    half-chunk drains per engine (21.6us on ACT) the out-phase critical
    path; 16 full-chunk drains per engine is ~12us, back under the PE,
    and it pulled the trailing output-DMA tail from ~10us to ~3.5us.
  * ~26us of 4-pass fp32 dummy matmuls on the idle PE during the
    collective dead zone keep the HAM duty-cycle governor from parking
    at k=4/8 (it clamps within ~3us of PE idle and the clamp persists
    into the out phase).

Sharding: S (sequence) split across 8 cores (128 positions each); each
core emits output columns for its own S-shard.
"""

import math

import numpy as np

import concourse.bass as bass
import concourse.mybir as mybir
import concourse.tile as tile
from concourse import bacc
from concourse.bass_utils import run_bass_kernel_spmd
from concourse.masks import make_identity

NCORES = 8
N = 128            # batch rows (attention is over this axis)
S = 1024           # sequence positions
D = 128            # feature dim
S_LOC = S // NCORES       # 128 positions per core
COLS = S_LOC * D          # 16384 free columns per core
NCH = S_LOC // 4          # 32 sweep chunks of 512 cols (4 positions)
VTAIL = 0                 # V fully fused into the sweep (the collective's
                          # start is gated by the NRT init barrier ~60us in,
                          # far after the sweep ends -- nothing to hide)
F32 = mybir.dt.float32
F32R = mybir.dt.float32r
F16 = mybir.dt.float16
BF16 = mybir.dt.bfloat16

_CACHE = {}


def _temporal_scale():
    """pe.flatten() from the reference's _temporal_pe, float32."""
    i = np.arange(32, dtype=np.float32)[:, None]
    j = np.arange(16, dtype=np.float32)[None, :]
    arg = (np.float32(1.0) * np.float32(np.pi) * i
           / np.power(np.float32(1000.0), (np.float32(2.0) * j / np.float32(128.0))))
    pe = np.stack([np.sin(arg), np.cos(arg)], axis=-1).reshape(32, 32)
    return pe.reshape(-1).astype(np.float32)   # [1024]


def _emit(nc, tc, xt_d, A_d, w_d, WvT_d, out_d):
    AX = mybir.AxisListType
    AF = mybir.ActivationFunctionType
    NFUSED = S_LOC - VTAIL          # positions with V fused into the sweep

    with (
        tc.tile_pool(name="consts", bufs=1) as consts,
        tc.tile_pool(name="xt", bufs=1) as xtp,
        tc.tile_pool(name="vbuf", bufs=1) as vp,
        tc.tile_pool(name="small", bufs=1) as small,
        tc.tile_pool(name="dram", bufs=1, space="DRAM") as dram,
    ):
        ident = consts.tile([128, 128], F32)
        make_identity(nc, ident[:])
        A_sb = consts.tile([D, D], F16)
        nc.sync.dma_start(A_sb[:], A_d[:])
        w_sb = consts.tile([D, 1], F32)
        nc.sync.dma_start(w_sb[:], w_d[:])
        WvT_sb = consts.tile([D, D], F16)
        nc.sync.dma_start(WvT_sb[:], WvT_d[:])

        XT = xtp.tile([128, COLS], F16)      # xs^T, [d, (s,n)]
        V = vp.tile([128, COLS], BF16)       # v rows, [m, (s,g)]

        scT_sb = small.tile([128, 128], F32, tag="scT")
        ar_sb = small.tile([128, 128], F32, tag="ar")
        exT = small.tile([128, 128], BF16, tag="exT")
        ones = small.tile([128, 2], BF16, tag="ones")
        rinv = small.tile([128, 1], F32, tag="rinv")
        nbias = small.tile([128, 1], F32, tag="nbias")
        nc.gpsimd.memset(ones[:], 1.0)
        nc.gpsimd.memset(nbias[:], -40.0)

        in_b = dram.tile([128, 128], F32)
        out_b = dram.tile([128, 128], F32)

        # XT input on one HWDGE queue, ascending so Y(0) starts early.
        bounds = [0, 128, 256, 512, 1024] + [1024 * i for i in range(2, 17)]
        for lo, hi in zip(bounds[:-1], bounds[1:]):
            nc.sync.dma_start(XT[:, lo:hi], xt_d[:, lo:hi])

        # Warm-up: PE clock gate starts at 1.2 GHz; burn transposes inside
        # the first-chunk DMA wait so the sweep starts warm.
        with tc.tile_pool(name="ps_wu", bufs=1, space="PSUM") as ps_wu:
            wps = ps_wu.tile([128, 128], F32)
            for _ in range(16):
                nc.tensor.transpose(wps[:], ident[:], ident[:])

        # ---- Sweep: Y = A^T@XT (+w), scT += XT_s^T@yt_s, V_s = XT_s^T@WvT
        with (
            tc.tile_pool(name="yt", bufs=3) as ytp,
            tc.tile_pool(name="ps_y", bufs=3, space="PSUM") as ps_y,
            tc.tile_pool(name="ps_v", bufs=2, space="PSUM") as ps_v,
            tc.tile_pool(name="ps_sc", bufs=1, space="PSUM") as ps_sc,
        ):
            sc_ps = ps_sc.tile([128, 128], F32)

            def emit_y(c):
                yps = ps_y.tile([128, 512], F32, tag="y")
                nc.tensor.matmul(yps[:], A_sb[:], XT[:, c * 512:(c + 1) * 512],
                                 start=True, stop=True)
                yt = ytp.tile([128, 512], F16, tag="yt")
                if c % 2 == 0:
                    nc.vector.tensor_scalar_add(yt[:], yps[:], w_sb[:, 0:1])
                else:
                    nc.scalar.add(yt[:], yps[:], w_sb[:, 0:1])
                return yt

            pending = emit_y(0)
            for c in range(NCH):
                yt = pending
                if c + 1 < NCH:
                    pending = emit_y(c + 1)
                vps = (ps_v.tile([128, 512], F32, tag="v", name="vps")
                       if 4 * c < NFUSED else None)
                for k in range(4):
                    s = 4 * c + k
                    xs_s = XT[:, s * 128:(s + 1) * 128]
                    nc.tensor.matmul(sc_ps[:], xs_s, yt[:, k * 128:(k + 1) * 128],
                                     start=(s == 0), stop=(s == S_LOC - 1))
                    if vps is not None:
                        nc.tensor.matmul(vps[:, k * 128:(k + 1) * 128], xs_s,
                                         WvT_sb[:], start=True, stop=True)
                if vps is not None:
                    dst = V[:, c * 512:(c + 1) * 512]
                    if c % 2 == 0:
                        nc.scalar.copy(dst, vps[:])
                    else:
                        nc.vector.tensor_copy(dst, vps[:])
            sc_done = nc.vector.tensor_copy(scT_sb[:], sc_ps[:])

        # ---- AllReduce the partial transposed scores (64 KiB) ----
        nc.sync.dma_start(in_b[:], scT_sb[:])
        nc.gpsimd.collective_compute(
            "AllReduce", mybir.AluOpType.add,
            replica_groups=[list(range(NCORES))],
            ins=[in_b[:].opt()], outs=[out_b[:].opt()],
        )
        nc.sync.dma_start(ar_sb[:, 0:64], out_b[:, 0:64])
        nc.scalar.dma_start(ar_sb[:, 64:128], out_b[:, 64:128])

        # ---- HAM-governor coaxing during the collective dead zone.  The
        # sweep's sustained single-pass matmuls clamp the PE to k=4/8 duty
        # and the clamp persists through idle, so the out phase would start
        # at half clock.  Sustained 4-pass fp32 matmul provably (baseline
        # phase 2) moves the governor back to k=8/8: burn ~26us of dummy
        # fp32 matmuls on the otherwise-idle PE so the out phase begins at
        # full clock.  Sized to end (~70us) before the collective typically
        # completes (~80-95us).
        with (
            tc.tile_pool(name="dummy", bufs=1) as dmp,
            tc.tile_pool(name="ps_dm", bufs=1, space="PSUM") as ps_dm,
        ):
            dmv = dmp.tile([128, 512], F32, tag="dmv")
            nc.gpsimd.memset(dmv[:], 0.0)
            dps = ps_dm.tile([128, 512], F32, tag="dps")
            for i in range(30):
                dm = nc.tensor.matmul(dps[:], ident[:], dmv[:],
                                      start=True, stop=True)
                if i == 0:
                    tile.add_dep_helper(dm.ins, sc_done.ins, sync=True,
                                        reason="dummies after score drain")

        # ---- V tail: hides the collective. Pinned after the score drain so
        # the scheduler cannot front-run it into the sweep.
        with tc.tile_pool(name="ps_v2", bufs=2, space="PSUM") as ps_v2:
            for c in range(NFUSED // 4, NCH):
                vps = ps_v2.tile([128, 512], F32, tag="v2")
                for k in range(4):
                    s = 4 * c + k
                    vm = nc.tensor.matmul(vps[:, k * 128:(k + 1) * 128],
                                          XT[:, s * 128:(s + 1) * 128],
                                          WvT_sb[:], start=True, stop=True)
                    if s == NFUSED:
                        tile.add_dep_helper(vm.ins, sc_done.ins, sync=True,
                                            reason="V tail after score drain")
                dst = V[:, c * 512:(c + 1) * 512]
                if c % 2 == 0:
                    nc.vector.tensor_copy(dst, vps[:])
                else:
                    nc.scalar.copy(dst, vps[:])

        # ---- softmax without transposes: exp stays in the scT [m, n]
        # layout (softmax-invariant constant -40 bias instead of a row-max;
        # logits for this problem are < ~75 so f32 exp cannot overflow).
        # The unnormalized exp^T IS the out-matmul stationary; the row sums
        # Z[n] come from a ones-matmul + 1-col transpose-matmul side chain
        # that overlaps the first out chunks, and 1/Z folds into the output
        # drain scaling.
        nc.scalar.activation(exT[:], ar_sb[:], AF.Exp, bias=nbias[:, 0:1],
                             scale=1.0)

        # ---- out = exT^T @ V * (1/Z), streamed to DRAM ----
        with (
            tc.tile_pool(name="osb", bufs=8) as osbp,
            tc.tile_pool(name="ps_z", bufs=1, space="PSUM") as ps_z,
            tc.tile_pool(name="ps_o", bufs=6, space="PSUM") as ps_o,
        ):
            ztp = ps_z.tile([128, 2], F32, tag="ztp")
            nc.tensor.matmul(ztp[:], exT[:], ones[:],
                             start=True, stop=True)
            nc.vector.reciprocal(rinv[:], ztp[:, 0:1])
            for c in range(NCH):
                ops = ps_o.tile([128, 512], F32, tag="o")
                nc.tensor.matmul(ops[:], exT[:], V[:, c * 512:(c + 1) * 512],
                                 start=True, stop=True)
                osb = osbp.tile([128, 512], F16, tag="osb")
                # whole-chunk drains, engines alternating by chunk: the
                # ~260ns fixed cost per drain instruction made 32 half-chunk
                # drains per engine (21.6us on ACT) the out-phase critical
                # path; 16 full-chunk drains per engine is 14.4us, back
                # under the PE.
                if c % 2 == 0:
                    nc.vector.tensor_scalar_mul(osb[:], ops[:], rinv[:, 0:1])
                else:
                    nc.scalar.mul(osb[:], ops[:], rinv[:, 0:1])
                # dma_start costs ~700ns of issue time on its engine's
                # stream, so keep output DMAs off ACT (busy with drains);
                # sync and gpsimd are otherwise idle here.
                eng = [nc.sync, nc.gpsimd][c % 2]
                eng.dma_start(out_d[:, c * 512:(c + 1) * 512], osb[:])


def _build():
    key = "v3"
    if key in _CACHE:
        return _CACHE[key]
    nc = bacc.Bacc("TRN2", target_bir_lowering=False, debug=False,
                   num_devices=NCORES)
    xt_d = nc.dram_tensor("xt", [128, COLS], F16, kind="ExternalInput")
    A_d = nc.dram_tensor("A", [D, D], F16, kind="ExternalInput")
    w_d = nc.dram_tensor("w", [D, 1], F32, kind="ExternalInput")
    WvT_d = nc.dram_tensor("WvT", [D, D], F16, kind="ExternalInput")
    out_d = nc.dram_tensor("out", [N, COLS], F16, kind="ExternalOutput")
    with tile.TileContext(nc) as tc:
        _emit(nc, tc, xt_d, A_d, w_d, WvT_d, out_d)
    nc.compile()
    _CACHE[key] = nc
    return nc


def prepare_inputs(x, W, b):
    """Host-side prep: shard + transpose x over S, build derived matrices."""
    x = np.asarray(x, dtype=np.float32)
    W = np.asarray(W, dtype=np.float32)
    b = np.asarray(b, dtype=np.float32)

    rs = math.sqrt(float(D))
    Wq = W[0::3, :].astype(np.float64) / rs
    Wk = W[1::3, :].astype(np.float64)
    Wv = W[2::3, :]
    bq = b[0::3].astype(np.float64) / rs
    bv = b[2::3]

    A = (Wq.T @ Wk).astype(np.float16)                       # [128, 128]
    w = (Wk.T @ bq).astype(np.float32)[:, None]              # [128, 1]
    WvT = np.ascontiguousarray(Wv.T).astype(np.float16)      # [128, 128]

    scale = _temporal_scale()                                # [1024]
    in_maps = []
    for c in range(NCORES):
        sl = slice(c * S_LOC, (c + 1) * S_LOC)
        xs_c = x[:, sl, :] * scale[sl][None, :, None]        # [n, s, d] f32
        xt_c = np.ascontiguousarray(
            xs_c.transpose(2, 1, 0)).reshape(D, COLS).astype(np.float16)
        in_maps.append({
            "xt": xt_c, "A": A, "w": w, "WvT": WvT,
        })
    return in_maps, bv


def run(inputs, trace=False, **kw):
    nc = _build()
    in_maps, bv = prepare_inputs(inputs["x"], inputs["W"], inputs["b"])
    res = run_bass_kernel_spmd(nc, in_maps, core_ids=list(range(NCORES)),
                               trace=trace, **kw)
    out = np.concatenate(
        [res.results[c]["out"].astype(np.float32) for c in range(NCORES)], axis=1)
    out += np.tile(bv, S)[None, :]     # v-bias: attn rows sum to 1
    return out, res


def kernel(x, W, b):
    out, _ = run({"x": x, "W": W, "b": b})
    return out
